# revision 41
# baseline (speedup 1.0000x reference)
"""Trainium2 Bass kernel for the EnrichClassifier pathway MLP (fp8 edition).

Network (eval mode, BN folded into weights):
  h1 = relu(x @ (w1*m1).T * s1 + b1')   [8192,5000] -> [8192,4000]
  h2 = relu(h1 @ (w2*m2).T * s2 + b2')                 -> [8192,2000]
  h3 = relu(h2 @ (w3*m3).T * s3 + b3')                 -> [8192,1000]
  sc = relu(h3 @ (w4*m4).T + b4)                       -> [8192,200]
  out = sc @ wc.T + bc                                 -> [8192,50]

Structure: m1 gives each of 200 pathways a private set of 100 genes;
20 L1 units per pathway share that set. m2/m3/m4 are block-diagonal
(20->10->5->1 per pathway). Per pathway the kernel gathers the 100
gene rows of x^T from DRAM (dma_gather, fp8) and runs tiny dense
per-pathway matmuls packed into the PE array.

fp8 scheme: x, h1..h3 and w1..w4 are e4m3; matmuls use DoubleRow
perf mode (2 k-slots of 128, 0.5 cyc/row) pairing two pathways (L1)
or two tiles (L2..L4) per instruction. The last stage (scores +
classifier) runs in fp16 to protect output precision. Weights w1/wc
and all intermediates carry a x16 scale to avoid fp8 subnormals;
the final activation rescales by 1/256.

Sharding: pure data parallel over batch across the 8 cores (1024 rows
per core); packed weights replicated.
"""

import contextlib

import numpy as np
import ml_dtypes

import concourse.bass as bass
import concourse.bacc as bacc
import concourse.tile as tile
import concourse.mybir as mybir
from concourse.bass_utils import run_bass_kernel_spmd

# ---------------- hardcoded geometry ----------------
B, G, NPATH = 8192, 5000, 200
NCORES = 8
BC = B // NCORES            # 1024 rows per core
NT = 2                      # batch tiles per core
NB = BC // NT               # 512 = PSUM bank free size (fp32)
U1, U2, U3 = 20, 10, 5      # per-pathway units per layer
NL = 50                     # labels
KPAD = 128                  # gene slots per pathway (padded)
KROW = 100                  # real genes per pathway (transfer/contraction rows)
SGS = 12                    # pathways per supergroup
NSG = 17                    # supergroups (16 full + 1 of 8)
NQUAD = 50                  # h1 tiles (4 pathways each)
NPAIR1 = NPATH // 2         # 100 L1 pathway pairs
NPAIR = 9                   # h3 tiles (24 pathways each, last 8)
NIDX = NPATH * KPAD         # 25600 gather slots
SCL = 16.0                  # fp8 anti-subnormal weight scale
F32 = mybir.dt.float32
F32R = mybir.dt.float32r
F16 = mybir.dt.float16
F8 = mybir.dt.float8e4
E4NP = ml_dtypes.float8_e4m3
RELU = mybir.ActivationFunctionType.Relu
IDENT = mybir.ActivationFunctionType.Identity
DR = mybir.MatmulPerfMode.DoubleRow

_COMPILED = None  # cached nc across calls


def _sg_paths(sg):
    return range(SGS * sg, min(SGS * sg + SGS, NPATH))


def _pack(inputs):
    """Host-side packing: BN folding, fp8 quantization, per-pathway weight
    blocks, gather index tables, per-core x^T slices."""
    f = lambda k: np.asarray(inputs[k], np.float32)
    x = f("x")
    w1, b1, m1 = f("w1"), f("b1"), f("m1")
    w2, b2, m2 = f("w2"), f("b2"), f("m2")
    w3, b3, m3 = f("w3"), f("b3"), f("m3")
    w4, b4, m4 = f("w4"), f("b4"), f("m4")
    wc, bc = f("wc"), f("bc")

    def fold(gamma, beta, rm, rv):
        s = gamma / np.sqrt(rv + 1e-5)
        return s, beta - rm * s

    s1, t1 = fold(f("gamma1"), f("beta1"), f("rm1"), f("rv1"))
    s2, t2 = fold(f("gamma2"), f("beta2"), f("rm2"), f("rv2"))
    s3, t3 = fold(f("gamma3"), f("beta3"), f("rm3"), f("rv3"))
    w1m = w1 * m1 * s1[:, None]
    b1f = (b1 * s1 + t1) * SCL
    w2m = w2 * m2 * s2[:, None]
    b2f = (b2 * s2 + t2) * SCL
    w3m = w3 * m3 * s3[:, None]
    b3f = (b3 * s3 + t3) * SCL
    w4m = w4 * m4
    b4f = b4 * SCL

    # per-pathway gene lists, padded to 128 with gene 0 (weight rows are 0
    # there anyway); slot q=pathway, partition p=padded gene position
    genes = []
    idx_all = np.zeros(NIDX, np.int64)
    for p in range(NPATH):
        g = np.nonzero(m1[U1 * p] != 0)[0]
        assert len(g) <= KROW
        genes.append(g)
        idx_all[KPAD * p : KPAD * p + len(g)] = g
    idx_mat = idx_all.reshape(NPATH, KPAD)  # [pathway, gene slot]

    # L1 stationary [128, NPAIR1, 2, 64] fp8 (x16): pair q = pathways
    # (2q, 2q+1); slot i = pathway 2q+i; col m: units at 0..19 / 32..51
    # so a quad's PSUM tile has its 4 pathways at 32-row pitch.
    w1s = np.zeros((KROW, NPAIR1, 2, 64), np.float32)
    b1v = np.zeros((128, NQUAD), np.float32)
    for p in range(NPATH):
        g = genes[p]
        q, i = divmod(p, 2)
        # within the pair block, member i sits at cols 32*i..32*i+20
        w1s[: len(g), q, i, 32 * i : 32 * i + U1] = (
            w1m[U1 * p : U1 * p + U1, g].T * SCL
        )
        t, j = divmod(p, 4)
        b1v[32 * j : 32 * j + U1, t] = b1f[U1 * p : U1 * p + U1]
    w1s = w1s.astype(E4NP)

    # L2 stationary [128, NSG, 3, 128] fp8: slots 0,1 = DoubleRow pair
    # (quads 3sg, 3sg+1), slot 2 = single quad 3sg+2 (absent for sg16).
    # rows 32j+u = h1 of pathway 4t+j ; cols 10l+v, l = sg-local path
    w2s = np.zeros((128, NSG, 3, 128), np.float32)
    b2v = np.zeros((128, NSG), np.float32)
    for t in range(NQUAD):
        sg = t // 3 if t < 48 else 16  # sgs 0..15 have quads 3sg..3sg+2; sg16 has 48,49
        slot = t - 3 * sg if t < 48 else t - 48
        for j in range(4):
            p = 4 * t + j
            l = p - SGS * (p // SGS)
            blk = w2m[U2 * p : U2 * p + U2, U1 * p : U1 * p + U1]  # [10,20]
            w2s[32 * j : 32 * j + U1, sg, slot, U2 * l : U2 * l + U2] = blk.T
    for sg in range(NSG):
        for l, p in enumerate(_sg_paths(sg)):
            b2v[U2 * l : U2 * l + U2, sg] = b2f[U2 * p : U2 * p + U2]
    w2s = w2s.astype(E4NP)

    # L3 stationary [128, NPAIR, 2, 128] fp8: pr<8 pairs sgs (2pr, 2pr+1)
    # in DR slots; pr=8 single (sg16) in slot 0.
    # rows 10l+v of slot's sg, cols 5q+w (q = pair-local path index)
    w3s = np.zeros((128, NPAIR, 2, 128), np.float32)
    b3v = np.zeros((128, NPAIR), np.float32)
    for sg in range(NSG):
        pr, slot = divmod(sg, 2)
        if sg == 16:
            pr, slot = 8, 0
        for l, p in enumerate(_sg_paths(sg)):
            qq = SGS * (sg % 2) + l
            blk = w3m[U3 * p : U3 * p + U3, U2 * p : U2 * p + U2]  # [5,10]
            w3s[U2 * l : U2 * l + U2, pr, slot, U3 * qq : U3 * qq + U3] = blk.T
    for pr in range(NPAIR):
        for p in range(24 * pr, min(24 * pr + 24, NPATH)):
            qq = p - 24 * pr
            b3v[U3 * qq : U3 * qq + U3, pr] = b3f[U3 * p : U3 * p + U3]
    w3s = w3s.astype(E4NP)

    # L4 stationary [128, 5, 2, 128] fp8: blocks 0=(pr0,pr1) 1=(pr2,pr3)
    # 2=(pr4,-) 3=(pr5,pr6) 4=(pr7,pr8); rows 5q+w, col 24*(pr%5)+q
    w4s = np.zeros((128, 5, 2, 128), np.float32)
    b4v = np.zeros((128, 2), np.float32)
    PRBLK = {0: (0, 0), 1: (0, 1), 2: (1, 0), 3: (1, 1), 4: (2, 0),
             5: (3, 0), 6: (3, 1), 7: (4, 0), 8: (4, 1)}
    for i in range(NPAIR):
        blk, slot = PRBLK[i]
        base = 24 * i if i < 5 else 24 * (i - 5)
        for p in range(24 * i, min(24 * i + 24, NPATH)):
            qq = p - 24 * i
            w4s[U3 * qq : U3 * qq + U3, blk, slot, base + qq] = w4m[p, U3 * p : U3 * p + U3]
    b4v[:120, 0] = b4f[:120]
    b4v[:80, 1] = b4f[120:]
    w4s = w4s.astype(E4NP)

    # classifier stationary [128, 2, 64] fp16 (x16): slot T rows = pathway
    # scores of group T, cols = labels
    wcs = np.zeros((128, 2, 64), np.float32)
    wcs[:120, 0, :NL] = wc[:, :120].T * SCL
    wcs[:80, 1, :NL] = wc[:, 120:].T * SCL
    wcs = wcs.astype(np.float16)
    bcv = np.zeros((128, 1), np.float32)
    bcv[:NL, 0] = bc

    ident = np.eye(64, dtype=np.float32)

    shared = {
        "w1s": w1s, "w2s": w2s, "w3s": w3s, "w4s": w4s, "wcs": wcs,
        "b1v": b1v, "b2v": b2v, "b3v": b3v, "b4v": b4v, "bcv": bcv,
        "ident": ident,
    }
    in_maps = []
    for c in range(NCORES):
        m = dict(shared)
        xc = x[BC * c : BC * (c + 1)].T.astype(E4NP)  # [5000, 1024] fp8
        # host-side row replication: slot q (pathway), partition p (gene
        # position) -> contiguous [128, NPATH*BC] so the per-sg load is a
        # plain contiguous DMA (128 descriptors x 12KB) instead of a
        # 1536-descriptor gather.
        rep = xc[idx_mat[:, :KROW], :]          # [NPATH, KROW, BC]
        m["xt"] = np.ascontiguousarray(
            rep.transpose(1, 0, 2).reshape(KROW, NPATH * BC))
        in_maps.append(m)
    return in_maps


def _build(repeat=None):
    """Build + compile the per-core Bass program (shared across cores).

    repeat: if set, wrap the whole compute body in an on-device For_i loop
    (used only for timing measurements; outputs are identical)."""
    nc = bacc.Bacc("TRN2", target_bir_lowering=False, debug=False,
                   enable_asserts=False)

    dram_in = {}
    for name, shape, dt_ in [
        ("xt", [KROW, NPATH * BC], F8),
        ("w1s", [KROW, NPAIR1, 2, 64], F8),
        ("w2s", [128, NSG, 3, 128], F8),
        ("w3s", [128, NPAIR, 2, 128], F8),
        ("w4s", [128, 5, 2, 128], F8),
        ("wcs", [128, 2, 64], F16),
        ("b1v", [128, NQUAD], F32), ("b2v", [128, NSG], F32),
        ("b3v", [128, NPAIR], F32), ("b4v", [128, 2], F32),
        ("bcv", [128, 1], F32), ("ident", [64, 64], F32),
    ]:
        dram_in[name] = nc.dram_tensor(name, shape, dt_, kind="ExternalInput").ap()
    out_d = nc.dram_tensor("out", [128, NT * 4 * 64], F32, kind="ExternalOutput").ap()

    with tile.TileContext(nc) as tc:
        const = tc.alloc_tile_pool(name="const", bufs=1, space="SBUF")
        cs = {}
        for name, ap in dram_in.items():
            if name == "xt":
                continue  # gathers read x^T straight from DRAM
            t = const.tile(ap.shape, ap.dtype, name=f"c_{name}")
            nc.sync.dma_start(t[:], ap[:])
            cs[name] = t

        gpool = tc.alloc_tile_pool(name="gath", bufs=4, space="SBUF")
        h1p = tc.alloc_tile_pool(name="h1", bufs=6, space="SBUF")   # [128,2,NB]
        h1q = tc.alloc_tile_pool(name="h1q", bufs=4, space="SBUF")  # [128,NB]
        h2p = tc.alloc_tile_pool(name="h2", bufs=4, space="SBUF")
        h2q = tc.alloc_tile_pool(name="h2q", bufs=2, space="SBUF")
        h3p = tc.alloc_tile_pool(name="h3", bufs=4, space="SBUF")
        h3q = tc.alloc_tile_pool(name="h3q", bufs=2, space="SBUF")
        scp = tc.alloc_tile_pool(name="sc", bufs=3, space="SBUF")   # [128,2,NB] f16
        otp = tc.alloc_tile_pool(name="ot", bufs=2, space="SBUF")
        osb = tc.alloc_tile_pool(name="osb", bufs=2, space="SBUF")
        ps1 = tc.alloc_tile_pool(name="ps1", bufs=2, space="PSUM")
        ps2 = tc.alloc_tile_pool(name="ps2", bufs=2, space="PSUM")
        ps4 = tc.alloc_tile_pool(name="ps4", bufs=2, space="PSUM")
        psx = tc.alloc_tile_pool(name="psx", bufs=2, space="PSUM")  # p3/pc/pt

        loop = tc.For_i(0, repeat, 1) if repeat else contextlib.nullcontext()
        with loop:
            h2d = [None, None]
            h2s = [None, None]
            h3d = [None, None]
            h3s = [None, None]
            p4t = [None, None]
            scd = [None, None]
            def back(sg, nq, h1d_l, h1s_l):
                """L2..L4 + activations for a supergroup whose L1 already
                issued. Runs one sg behind L1 so the PE never waits on
                freshly-written h1 tiles."""
                for nt in range(NT):
                    h1d, h1s = h1d_l[nt], h1s_l[nt]
                    # ---- L2: DR over quad pair + single third quad ----
                    p2 = ps2.tile([128, NB], F32, name="p2", tag="p2")
                    nc.tensor.matmul(
                        p2[:], cs["w2s"][:, sg, 0:2, :], h1d[:],
                        start=True, stop=(nq == 2), perf_mode=DR,
                    )
                    if nq == 3:
                        nc.tensor.matmul(
                            p2[:], cs["w2s"][:, sg, 2, :], h1s[:],
                            start=False, stop=True,
                        )
                    # h2 into pair-tile halves (even sg: left, odd: right)
                    if sg % 2 == 0 and sg != 16:
                        if nt == 0:
                            h2d[0] = h2p.tile([128, 2, NB], F8, name="h2d", tag="h2d")
                            h2d[1] = h2p.tile([128, 2, NB], F8, name="h2e", tag="h2e")
                        h2dst = h2d[nt][:, 0, :]
                    elif sg != 16:
                        h2dst = h2d[nt][:, 1, :]
                    else:
                        h2s[nt] = h2q.tile([128, NB], F8, name="h2s", tag="h2s")
                        h2dst = h2s[nt][:]
                    if (sg + nt) % 2 == 1:
                        nc.scalar.activation(h2dst, p2[:], RELU,
                                             bias=cs["b2v"][:, sg : sg + 1])
                    else:
                        nc.vector.tensor_scalar(h2dst, p2[:],
                                                cs["b2v"][:, sg : sg + 1], 0.0,
                                                mybir.AluOpType.add,
                                                mybir.AluOpType.max)
                for nt in range(NT):
                    # ---- L3 per pair of supergroups ----
                    if sg % 2 == 1 or sg == NSG - 1:
                        pr = sg // 2
                        p3 = psx.tile([128, NB], F32, name="px", tag="px")
                        if sg != 16:
                            nc.tensor.matmul(
                                p3[:], cs["w3s"][:, pr, 0:2, :], h2d[nt][:],
                                start=True, stop=True, perf_mode=DR,
                            )
                        else:
                            nc.tensor.matmul(
                                p3[:], cs["w3s"][:, pr, 0, :], h2s[nt][:],
                                start=True, stop=True,
                            )
                        # h3 into pair tiles per L4 DR block structure
                        blk, slot = {0: (0, 0), 1: (0, 1), 2: (1, 0),
                                     3: (1, 1), 4: (2, 0), 5: (3, 0),
                                     6: (3, 1), 7: (4, 0), 8: (4, 1)}[pr]
                        if pr == 4:
                            h3s[nt] = h3q.tile([128, NB], F8, name="h3s", tag="h3s")
                            h3dst = h3s[nt][:]
                        elif slot == 0:
                            h3d[nt] = h3p.tile([128, 2, NB], F8, name="h3d", tag="h3d")
                            h3dst = h3d[nt][:, 0, :]
                        else:
                            h3dst = h3d[nt][:, 1, :]
                        if (pr + nt) % 2 == 0:
                            nc.scalar.activation(h3dst, p3[:], RELU,
                                                 bias=cs["b3v"][:, pr : pr + 1])
                        else:
                            nc.vector.tensor_scalar(h3dst, p3[:],
                                                    cs["b3v"][:, pr : pr + 1], 0.0,
                                                    mybir.AluOpType.add,
                                                    mybir.AluOpType.max)
                        # ---- L4: DR per h3 pair tile; groups A(0-4) B(5-8)
                        T = 0 if pr < 5 else 1
                        grp_end = pr in (4, NPAIR - 1)
                        if pr in (1, 6):
                            p4t[nt] = ps4.tile([128, NB], F32, name="p4", tag="p4")
                        if pr == 4:
                            nc.tensor.matmul(
                                p4t[nt][:], cs["w4s"][:, blk, 0, :], h3s[nt][:],
                                start=False, stop=True,
                            )
                        elif slot == 1:
                            nc.tensor.matmul(
                                p4t[nt][:], cs["w4s"][:, blk, 0:2, :], h3d[nt][:],
                                start=(pr in (1, 6)), stop=(pr == 8),
                                perf_mode=DR,
                            )
                        if grp_end:
                            if T == 0:
                                scd[nt] = scp.tile([128, 2, NB], F16, name="scd", tag="scd")
                            nc.scalar.activation(scd[nt][:, T, :], p4t[nt][:], RELU,
                                                 bias=cs["b4v"][:, T : T + 1])

            prev = None
            for sg in range(NSG):
                npth = len(_sg_paths(sg))
                nq = (npth + 3) // 4
                # ---- load the sg's pre-replicated gene rows (fp8): plain
                # contiguous DMA, 100 descriptors x npth KB ----
                gtc = gpool.tile([KROW, npth, BC], F8, name="gt", tag="gt")
                src = dram_in["xt"][:, BC * SGS * sg : BC * (SGS * sg + npth)]
                nc.sync.dma_start(gtc[:], src.rearrange("p (s c) -> p s c", c=BC))
                gb = 0
                # ---- L1 for BOTH batch tiles (PE queue stays fed while
                # activations drain); back layers run one sg behind ----
                h1d_l = [None, None]
                h1s_l = [None, None]
                for nt in range(NT):
                    cab = slice(NB * nt, NB * (nt + 1))
                    # L1 per quad: DR pair at partitions 0-63 (ISA: DR dst
                    # must start at 0) + two non-DR fp8 matmuls for the
                    # other pair at 64-95 / 96-127
                    for g in range(nq):
                        t = 3 * sg + g if sg < 16 else 48 + g
                        p1 = ps1.tile([128, NB], F32, name="p1", tag="p1")
                        q0 = 6 * sg + 2 * g  # DR pair (pathways 4t, 4t+1)
                        nc.tensor.matmul(
                            p1[0:64, :], cs["w1s"][:, q0],
                            gtc[:, gb + 4 * g : gb + 4 * g + 2, cab],
                            start=True, stop=True, perf_mode=DR,
                            tile_position=(0, 0),
                        )
                        for i in range(2):  # pathways 4t+2, 4t+3
                            nc.tensor.matmul(
                                p1[64 + 32 * i : 96 + 32 * i, :],
                                cs["w1s"][:, q0 + 1, i, 32 * i : 32 * i + 32],
                                gtc[:, gb + 4 * g + 2 + i, cab],
                                start=True, stop=True,
                                tile_position=(0, 64 + 32 * i),
                            )
                        bias = cs["b1v"][:, t : t + 1]
                        if g < 2:
                            if g == 0:
                                h1d_l[nt] = h1p.tile([128, 2, NB], F8, name="h1d", tag="h1d")
                            dst = h1d_l[nt][:, g, :]
                        else:
                            h1s_l[nt] = h1q.tile([128, NB], F8, name="h1s", tag="h1s")
                            dst = h1s_l[nt][:]
                        if (t + nt) % 2 == 0:
                            nc.scalar.activation(dst, p1[:], RELU, bias=bias)
                        else:
                            nc.vector.tensor_scalar(dst, p1[:], bias, 0.0,
                                                    mybir.AluOpType.add,
                                                    mybir.AluOpType.max)
                if prev is not None:
                    back(*prev)
                prev = (sg, nq, h1d_l, h1s_l)
            back(*prev)
            # ---- classifier (fp16; scores are fp16) ----
            for nt in range(NT):
                pc = ps2.tile([128, NB], F32, name="p2", tag="p2")
                for T in range(2):
                    nc.tensor.matmul(
                        pc[:64, :], cs["wcs"][:, T, :], scd[nt][:, T, :],
                        start=(T == 0), stop=(T == 1),
                    )
                ot = otp.tile([64, NB], F32, name="ott", tag="ott")
                nc.scalar.activation(ot[:], pc[:64, :], IDENT,
                                     bias=cs["bcv"][:64, 0:1],
                                     scale=1.0 / (SCL * SCL))
                # ---- transpose [64, 512] -> 4 x [128, 64] and store ----
                ob = osb.tile([128, 4 * 64], F32, name="obt", tag="obt")
                for c in range(4):
                    pt = psx.tile([128, NB], F32, name="px", tag="px")
                    nc.tensor.transpose(pt[:, 0:64], ot[:, 128 * c : 128 * (c + 1)],
                                        cs["ident"][:])
                    nc.vector.tensor_copy(ob[:, 64 * c : 64 * (c + 1)], pt[:, 0:64])
                nc.sync.dma_start(
                    out_d[:, 4 * 64 * nt : 4 * 64 * (nt + 1)], ob[:])

        for pl in (psx, ps4, ps2, ps1, osb, otp, scp,
                   h3q, h3p, h2q, h2p, h1q, h1p, gpool, const):
            pl.release()

    nc.compile()
    return nc


def get_compiled():
    global _COMPILED
    if _COMPILED is None:
        _COMPILED = _build()
    return _COMPILED


def kernel(**inputs):
    nc = get_compiled()
    in_maps = _pack(inputs)
    res = run_bass_kernel_spmd(nc, in_maps, core_ids=list(range(NCORES)))
    outs = []
    for c in range(NCORES):
        ob = res.results[c]["out"].reshape(128, NT, 4, 64)
        outs.append(ob.transpose(1, 2, 0, 3).reshape(BC, 64)[:, :NL])
    return np.ascontiguousarray(np.concatenate(outs, axis=0))


if __name__ == "__main__":
    rng = np.random.default_rng(0)
    fake = {"x": rng.standard_normal((B, G), dtype=np.float32)}
    print("built", get_compiled())


# revision 42
# speedup vs baseline: 1.3178x; 1.3178x over previous
"""Trainium2 Bass kernel for the EnrichClassifier pathway MLP (fp8 edition).

Network (eval mode, BN folded into weights):
  h1 = relu(x @ (w1*m1).T * s1 + b1')   [8192,5000] -> [8192,4000]
  h2 = relu(h1 @ (w2*m2).T * s2 + b2')                 -> [8192,2000]
  h3 = relu(h2 @ (w3*m3).T * s3 + b3')                 -> [8192,1000]
  sc = relu(h3 @ (w4*m4).T + b4)                       -> [8192,200]
  out = sc @ wc.T + bc                                 -> [8192,50]

Structure: m1 gives each of 200 pathways a private set of 100 genes;
20 L1 units per pathway share that set. m2/m3/m4 are block-diagonal
(20->10->5->1 per pathway). Per pathway the kernel gathers the 100
gene rows of x^T from DRAM (dma_gather, fp8) and runs tiny dense
per-pathway matmuls packed into the PE array.

fp8 scheme: x, h1..h3 and w1..w4 are e4m3; matmuls use DoubleRow
perf mode (2 k-slots of 128, 0.5 cyc/row) pairing two pathways (L1)
or two tiles (L2..L4) per instruction. The last stage (scores +
classifier) runs in fp16 to protect output precision. Weights w1/wc
and all intermediates carry a x16 scale to avoid fp8 subnormals;
the final activation rescales by 1/256.

Sharding: pure data parallel over batch across the 8 cores (1024 rows
per core); packed weights replicated.
"""

import contextlib

import numpy as np
import ml_dtypes

import concourse.bass as bass
import concourse.bacc as bacc
import concourse.tile as tile
import concourse.mybir as mybir
from concourse.bass_utils import run_bass_kernel_spmd

# ---------------- hardcoded geometry ----------------
B, G, NPATH = 8192, 5000, 200
NCORES = 8
BC = B // NCORES            # 1024 rows per core
NT = 2                      # batch tiles per core
NB = BC // NT               # 512 = PSUM bank free size (fp32)
U1, U2, U3 = 20, 10, 5      # per-pathway units per layer
NL = 50                     # labels
KPAD = 128                  # gene slots per pathway (padded)
KROW = 100                  # real genes per pathway (transfer/contraction rows)
SGS = 12                    # pathways per supergroup
NSG = 17                    # supergroups (16 full + 1 of 8)
NQUAD = 50                  # h1 tiles (4 pathways each)
NPAIR1 = NPATH // 2         # 100 L1 pathway pairs
NPAIR = 9                   # h3 tiles (24 pathways each, last 8)
NIDX = NPATH * KPAD         # 25600 gather slots
SCL = 16.0                  # fp8 anti-subnormal weight scale
F32 = mybir.dt.float32
F32R = mybir.dt.float32r
F16 = mybir.dt.float16
F8 = mybir.dt.float8e4
E4NP = ml_dtypes.float8_e4m3
RELU = mybir.ActivationFunctionType.Relu
IDENT = mybir.ActivationFunctionType.Identity
DR = mybir.MatmulPerfMode.DoubleRow

_COMPILED = None  # cached nc across calls


def _sg_paths(sg):
    return range(SGS * sg, min(SGS * sg + SGS, NPATH))


def _pack(inputs):
    """Host-side packing: BN folding, fp8 quantization, per-pathway weight
    blocks, gather index tables, per-core x^T slices."""
    f = lambda k: np.asarray(inputs[k], np.float32)
    x = f("x")
    w1, b1, m1 = f("w1"), f("b1"), f("m1")
    w2, b2, m2 = f("w2"), f("b2"), f("m2")
    w3, b3, m3 = f("w3"), f("b3"), f("m3")
    w4, b4, m4 = f("w4"), f("b4"), f("m4")
    wc, bc = f("wc"), f("bc")

    def fold(gamma, beta, rm, rv):
        s = gamma / np.sqrt(rv + 1e-5)
        return s, beta - rm * s

    s1, t1 = fold(f("gamma1"), f("beta1"), f("rm1"), f("rv1"))
    s2, t2 = fold(f("gamma2"), f("beta2"), f("rm2"), f("rv2"))
    s3, t3 = fold(f("gamma3"), f("beta3"), f("rm3"), f("rv3"))
    w1m = w1 * m1 * s1[:, None]
    b1f = (b1 * s1 + t1) * SCL
    w2m = w2 * m2 * s2[:, None]
    b2f = (b2 * s2 + t2) * SCL
    w3m = w3 * m3 * s3[:, None]
    b3f = (b3 * s3 + t3) * SCL
    w4m = w4 * m4
    b4f = b4 * SCL

    # per-pathway gene lists, padded to 128 with gene 0 (weight rows are 0
    # there anyway); slot q=pathway, partition p=padded gene position
    genes = []
    idx_all = np.zeros(NIDX, np.int64)
    for p in range(NPATH):
        g = np.nonzero(m1[U1 * p] != 0)[0]
        assert len(g) <= KROW
        genes.append(g)
        idx_all[KPAD * p : KPAD * p + len(g)] = g
    idx_mat = idx_all.reshape(NPATH, KPAD)  # [pathway, gene slot]

    # L1 stationary [128, NPAIR1, 2, 64] fp8 (x16): pair q = pathways
    # (2q, 2q+1); slot i = pathway 2q+i; col m: units at 0..19 / 32..51
    # so a quad's PSUM tile has its 4 pathways at 32-row pitch.
    w1s = np.zeros((KROW, NPAIR1, 2, 64), np.float32)
    b1v = np.zeros((128, NQUAD), np.float32)
    for p in range(NPATH):
        g = genes[p]
        q, i = divmod(p, 2)
        # within the pair block, member i sits at cols 32*i..32*i+20
        w1s[: len(g), q, i, 32 * i : 32 * i + U1] = (
            w1m[U1 * p : U1 * p + U1, g].T * SCL
        )
        t, j = divmod(p, 4)
        b1v[32 * j : 32 * j + U1, t] = b1f[U1 * p : U1 * p + U1]
    w1s = w1s.astype(E4NP)

    # L2 stationary [128, NSG, 3, 128] fp8: slots 0,1 = DoubleRow pair
    # (quads 3sg, 3sg+1), slot 2 = single quad 3sg+2 (absent for sg16).
    # rows 32j+u = h1 of pathway 4t+j ; cols 10l+v, l = sg-local path
    w2s = np.zeros((128, NSG, 3, 128), np.float32)
    b2v = np.zeros((128, NSG), np.float32)
    for t in range(NQUAD):
        sg = t // 3 if t < 48 else 16  # sgs 0..15 have quads 3sg..3sg+2; sg16 has 48,49
        slot = t - 3 * sg if t < 48 else t - 48
        for j in range(4):
            p = 4 * t + j
            l = p - SGS * (p // SGS)
            blk = w2m[U2 * p : U2 * p + U2, U1 * p : U1 * p + U1]  # [10,20]
            w2s[32 * j : 32 * j + U1, sg, slot, U2 * l : U2 * l + U2] = blk.T
    for sg in range(NSG):
        for l, p in enumerate(_sg_paths(sg)):
            b2v[U2 * l : U2 * l + U2, sg] = b2f[U2 * p : U2 * p + U2]
    w2s = w2s.astype(E4NP)

    # L3 stationary [128, NPAIR, 2, 128] fp8: pr<8 pairs sgs (2pr, 2pr+1)
    # in DR slots; pr=8 single (sg16) in slot 0.
    # rows 10l+v of slot's sg, cols 5q+w (q = pair-local path index)
    w3s = np.zeros((128, NPAIR, 2, 128), np.float32)
    b3v = np.zeros((128, NPAIR), np.float32)
    for sg in range(NSG):
        pr, slot = divmod(sg, 2)
        if sg == 16:
            pr, slot = 8, 0
        for l, p in enumerate(_sg_paths(sg)):
            qq = SGS * (sg % 2) + l
            blk = w3m[U3 * p : U3 * p + U3, U2 * p : U2 * p + U2]  # [5,10]
            w3s[U2 * l : U2 * l + U2, pr, slot, U3 * qq : U3 * qq + U3] = blk.T
    for pr in range(NPAIR):
        for p in range(24 * pr, min(24 * pr + 24, NPATH)):
            qq = p - 24 * pr
            b3v[U3 * qq : U3 * qq + U3, pr] = b3f[U3 * p : U3 * p + U3]
    w3s = w3s.astype(E4NP)

    # L4 stationary [128, 5, 2, 128] fp8: blocks 0=(pr0,pr1) 1=(pr2,pr3)
    # 2=(pr4,-) 3=(pr5,pr6) 4=(pr7,pr8); rows 5q+w, col 24*(pr%5)+q
    w4s = np.zeros((128, 5, 2, 128), np.float32)
    b4v = np.zeros((128, 2), np.float32)
    PRBLK = {0: (0, 0), 1: (0, 1), 2: (1, 0), 3: (1, 1), 4: (2, 0),
             5: (3, 0), 6: (3, 1), 7: (4, 0), 8: (4, 1)}
    for i in range(NPAIR):
        blk, slot = PRBLK[i]
        base = 24 * i if i < 5 else 24 * (i - 5)
        for p in range(24 * i, min(24 * i + 24, NPATH)):
            qq = p - 24 * i
            w4s[U3 * qq : U3 * qq + U3, blk, slot, base + qq] = w4m[p, U3 * p : U3 * p + U3]
    b4v[:120, 0] = b4f[:120]
    b4v[:80, 1] = b4f[120:]
    w4s = w4s.astype(E4NP)

    # classifier stationary [128, 2, 64] fp16 (x16): slot T rows = pathway
    # scores of group T, cols = labels
    wcs = np.zeros((128, 2, 64), np.float32)
    wcs[:120, 0, :NL] = wc[:, :120].T * SCL
    wcs[:80, 1, :NL] = wc[:, 120:].T * SCL
    wcs = wcs.astype(np.float16)
    bcv = np.zeros((128, 1), np.float32)
    bcv[:NL, 0] = bc

    ident = np.eye(64, dtype=np.float32)

    shared = {
        "w1s": w1s, "w2s": w2s, "w3s": w3s, "w4s": w4s, "wcs": wcs,
        "b1v": b1v, "b2v": b2v, "b3v": b3v, "b4v": b4v, "bcv": bcv,
        "ident": ident,
    }
    in_maps = []
    for c in range(NCORES):
        m = dict(shared)
        xc = x[BC * c : BC * (c + 1)].T.astype(E4NP)  # [5000, 1024] fp8
        # host-side row replication: slot q (pathway), partition p (gene
        # position) -> contiguous [128, NPATH*BC] so the per-sg load is a
        # plain contiguous DMA (128 descriptors x 12KB) instead of a
        # 1536-descriptor gather.
        rep = xc[idx_mat[:, :KROW], :]          # [NPATH, KROW, BC]
        m["xt"] = np.ascontiguousarray(
            rep.transpose(1, 0, 2).reshape(KROW, NPATH * BC))
        in_maps.append(m)
    return in_maps


def _build(repeat=None):
    """Build + compile the per-core Bass program (shared across cores).

    repeat: if set, wrap the whole compute body in an on-device For_i loop
    (used only for timing measurements; outputs are identical)."""
    nc = bacc.Bacc("TRN2", target_bir_lowering=False, debug=False,
                   enable_asserts=False)

    dram_in = {}
    for name, shape, dt_ in [
        ("xt", [KROW, NPATH * BC], F8),
        ("w1s", [KROW, NPAIR1, 2, 64], F8),
        ("w2s", [128, NSG, 3, 128], F8),
        ("w3s", [128, NPAIR, 2, 128], F8),
        ("w4s", [128, 5, 2, 128], F8),
        ("wcs", [128, 2, 64], F16),
        ("b1v", [128, NQUAD], F32), ("b2v", [128, NSG], F32),
        ("b3v", [128, NPAIR], F32), ("b4v", [128, 2], F32),
        ("bcv", [128, 1], F32), ("ident", [64, 64], F32),
    ]:
        dram_in[name] = nc.dram_tensor(name, shape, dt_, kind="ExternalInput").ap()
    out_d = nc.dram_tensor("out", [128, NT * 4 * 64], F32, kind="ExternalOutput").ap()

    with tile.TileContext(nc) as tc:
        const = tc.alloc_tile_pool(name="const", bufs=1, space="SBUF")
        cs = {}
        for name, ap in dram_in.items():
            if name == "xt":
                continue  # gathers read x^T straight from DRAM
            t = const.tile(ap.shape, ap.dtype, name=f"c_{name}")
            nc.sync.dma_start(t[:], ap[:])
            cs[name] = t

        gpool = tc.alloc_tile_pool(name="gath", bufs=4, space="SBUF")
        h1p = tc.alloc_tile_pool(name="h1", bufs=6, space="SBUF")   # [128,2,NB]
        h1q = tc.alloc_tile_pool(name="h1q", bufs=4, space="SBUF")  # [128,NB]
        h2p = tc.alloc_tile_pool(name="h2", bufs=4, space="SBUF")
        h2q = tc.alloc_tile_pool(name="h2q", bufs=2, space="SBUF")
        h3p = tc.alloc_tile_pool(name="h3", bufs=4, space="SBUF")
        h3q = tc.alloc_tile_pool(name="h3q", bufs=2, space="SBUF")
        scp = tc.alloc_tile_pool(name="sc", bufs=3, space="SBUF")   # [128,2,NB] f16
        otp = tc.alloc_tile_pool(name="ot", bufs=2, space="SBUF")
        osb = tc.alloc_tile_pool(name="osb", bufs=2, space="SBUF")
        ps1 = tc.alloc_tile_pool(name="ps1", bufs=3, space="PSUM")
        ps2 = tc.alloc_tile_pool(name="ps2", bufs=2, space="PSUM")
        ps4 = tc.alloc_tile_pool(name="ps4", bufs=2, space="PSUM")
        psx = tc.alloc_tile_pool(name="psx", bufs=1, space="PSUM")  # p3/pc/pt

        loop = tc.For_i(0, repeat, 1) if repeat else contextlib.nullcontext()
        with loop:
            h2d = [None, None]
            h2s = [None, None]
            h3d = [None, None]
            h3s = [None, None]
            p4t = [None, None]
            scd = [None, None]
            def back(sg, nq, h1d_l, h1s_l):
                """L2..L4 + activations for a supergroup whose L1 already
                issued. Runs one sg behind L1 so the PE never waits on
                freshly-written h1 tiles."""
                for nt in range(NT):
                    h1d, h1s = h1d_l[nt], h1s_l[nt]
                    # ---- L2: DR over quad pair + single third quad ----
                    p2 = ps2.tile([128, NB], F32, name="p2", tag="p2")
                    nc.tensor.matmul(
                        p2[:], cs["w2s"][:, sg, 0:2, :], h1d[:],
                        start=True, stop=(nq == 2), perf_mode=DR,
                    )
                    if nq == 3:
                        nc.tensor.matmul(
                            p2[:], cs["w2s"][:, sg, 2, :], h1s[:],
                            start=False, stop=True,
                        )
                    # h2 into pair-tile halves (even sg: left, odd: right)
                    if sg % 2 == 0 and sg != 16:
                        if nt == 0:
                            h2d[0] = h2p.tile([128, 2, NB], F8, name="h2d", tag="h2d")
                            h2d[1] = h2p.tile([128, 2, NB], F8, name="h2e", tag="h2e")
                        h2dst = h2d[nt][:, 0, :]
                    elif sg != 16:
                        h2dst = h2d[nt][:, 1, :]
                    else:
                        h2s[nt] = h2q.tile([128, NB], F8, name="h2s", tag="h2s")
                        h2dst = h2s[nt][:]
                    if (sg + nt) % 2 == 1:
                        nc.scalar.activation(h2dst, p2[:], RELU,
                                             bias=cs["b2v"][:, sg : sg + 1])
                    else:
                        nc.vector.tensor_scalar(h2dst, p2[:],
                                                cs["b2v"][:, sg : sg + 1], 0.0,
                                                mybir.AluOpType.add,
                                                mybir.AluOpType.max)
                for nt in range(NT):
                    # ---- L3 per pair of supergroups ----
                    if sg % 2 == 1 or sg == NSG - 1:
                        pr = sg // 2
                        p3 = psx.tile([128, NB], F32, name="px", tag="px")
                        if sg != 16:
                            nc.tensor.matmul(
                                p3[:], cs["w3s"][:, pr, 0:2, :], h2d[nt][:],
                                start=True, stop=True, perf_mode=DR,
                            )
                        else:
                            nc.tensor.matmul(
                                p3[:], cs["w3s"][:, pr, 0, :], h2s[nt][:],
                                start=True, stop=True,
                            )
                        # h3 into pair tiles per L4 DR block structure
                        blk, slot = {0: (0, 0), 1: (0, 1), 2: (1, 0),
                                     3: (1, 1), 4: (2, 0), 5: (3, 0),
                                     6: (3, 1), 7: (4, 0), 8: (4, 1)}[pr]
                        if pr == 4:
                            h3s[nt] = h3q.tile([128, NB], F8, name="h3s", tag="h3s")
                            h3dst = h3s[nt][:]
                        elif slot == 0:
                            h3d[nt] = h3p.tile([128, 2, NB], F8, name="h3d", tag="h3d")
                            h3dst = h3d[nt][:, 0, :]
                        else:
                            h3dst = h3d[nt][:, 1, :]
                        if (pr + nt) % 2 == 0:
                            nc.scalar.activation(h3dst, p3[:], RELU,
                                                 bias=cs["b3v"][:, pr : pr + 1])
                        else:
                            nc.vector.tensor_scalar(h3dst, p3[:],
                                                    cs["b3v"][:, pr : pr + 1], 0.0,
                                                    mybir.AluOpType.add,
                                                    mybir.AluOpType.max)
                        # ---- L4: DR per h3 pair tile; groups A(0-4) B(5-8)
                        T = 0 if pr < 5 else 1
                        grp_end = pr in (4, NPAIR - 1)
                        if pr in (1, 6):
                            p4t[nt] = ps4.tile([128, NB], F32, name="p4", tag="p4")
                        if pr == 4:
                            nc.tensor.matmul(
                                p4t[nt][:], cs["w4s"][:, blk, 0, :], h3s[nt][:],
                                start=False, stop=True,
                            )
                        elif slot == 1:
                            nc.tensor.matmul(
                                p4t[nt][:], cs["w4s"][:, blk, 0:2, :], h3d[nt][:],
                                start=(pr in (1, 6)), stop=(pr == 8),
                                perf_mode=DR,
                            )
                        if grp_end:
                            if T == 0:
                                scd[nt] = scp.tile([128, 2, NB], F16, name="scd", tag="scd")
                            nc.scalar.activation(scd[nt][:, T, :], p4t[nt][:], RELU,
                                                 bias=cs["b4v"][:, T : T + 1])

            prev = None
            for sg in range(NSG):
                npth = len(_sg_paths(sg))
                nq = (npth + 3) // 4
                # ---- load the sg's pre-replicated gene rows (fp8): plain
                # contiguous DMA, 100 descriptors x npth KB ----
                gtc = gpool.tile([KROW, npth, BC], F8, name="gt", tag="gt")
                src = dram_in["xt"][:, BC * SGS * sg : BC * (SGS * sg + npth)]
                nc.sync.dma_start(gtc[:], src.rearrange("p (s c) -> p s c", c=BC))
                gb = 0
                # ---- L1 for BOTH batch tiles (PE queue stays fed while
                # activations drain); back layers run one sg behind ----
                h1d_l = [None, None]
                h1s_l = [None, None]
                for nt in range(NT):
                    cab = slice(NB * nt, NB * (nt + 1))
                    # L1 per quad: DR pair at partitions 0-63 (ISA: DR dst
                    # must start at 0) + two non-DR fp8 matmuls for the
                    # other pair at 64-95 / 96-127
                    for g in range(nq):
                        t = 3 * sg + g if sg < 16 else 48 + g
                        p1 = ps1.tile([128, NB], F32, name="p1", tag="p1")
                        q0 = 6 * sg + 2 * g  # DR pair (pathways 4t, 4t+1)
                        nc.tensor.matmul(
                            p1[0:64, :], cs["w1s"][:, q0],
                            gtc[:, gb + 4 * g : gb + 4 * g + 2, cab],
                            start=True, stop=True, perf_mode=DR,
                            tile_position=(0, 0),
                        )
                        for i in range(2):  # pathways 4t+2, 4t+3
                            nc.tensor.matmul(
                                p1[64 + 32 * i : 96 + 32 * i, :],
                                cs["w1s"][:, q0 + 1, i, 32 * i : 32 * i + 32],
                                gtc[:, gb + 4 * g + 2 + i, cab],
                                start=True, stop=True,
                                tile_position=(0, 64 + 32 * i),
                            )
                        bias = cs["b1v"][:, t : t + 1]
                        if g < 2:
                            if g == 0:
                                h1d_l[nt] = h1p.tile([128, 2, NB], F8, name="h1d", tag="h1d")
                            dst = h1d_l[nt][:, g, :]
                        else:
                            h1s_l[nt] = h1q.tile([128, NB], F8, name="h1s", tag="h1s")
                            dst = h1s_l[nt][:]
                        if (t + nt) % 2 == 0:
                            nc.scalar.activation(dst, p1[:], RELU, bias=bias)
                        else:
                            nc.vector.tensor_scalar(dst, p1[:], bias, 0.0,
                                                    mybir.AluOpType.add,
                                                    mybir.AluOpType.max)
                if prev is not None:
                    back(*prev)
                prev = (sg, nq, h1d_l, h1s_l)
            back(*prev)
            # ---- classifier (fp16; scores are fp16) ----
            for nt in range(NT):
                pc = ps2.tile([128, NB], F32, name="p2", tag="p2")
                for T in range(2):
                    nc.tensor.matmul(
                        pc[:64, :], cs["wcs"][:, T, :], scd[nt][:, T, :],
                        start=(T == 0), stop=(T == 1),
                    )
                ot = otp.tile([64, NB], F32, name="ott", tag="ott")
                nc.scalar.activation(ot[:], pc[:64, :], IDENT,
                                     bias=cs["bcv"][:64, 0:1],
                                     scale=1.0 / (SCL * SCL))
                # ---- transpose [64, 512] -> 4 x [128, 64] and store ----
                ob = osb.tile([128, 4 * 64], F32, name="obt", tag="obt")
                for c in range(4):
                    pt = psx.tile([128, NB], F32, name="px", tag="px")
                    nc.tensor.transpose(pt[:, 0:64], ot[:, 128 * c : 128 * (c + 1)],
                                        cs["ident"][:])
                    nc.vector.tensor_copy(ob[:, 64 * c : 64 * (c + 1)], pt[:, 0:64])
                nc.sync.dma_start(
                    out_d[:, 4 * 64 * nt : 4 * 64 * (nt + 1)], ob[:])

        for pl in (psx, ps4, ps2, ps1, osb, otp, scp,
                   h3q, h3p, h2q, h2p, h1q, h1p, gpool, const):
            pl.release()

    nc.compile()
    return nc


def get_compiled():
    global _COMPILED
    if _COMPILED is None:
        _COMPILED = _build()
    return _COMPILED


def kernel(**inputs):
    nc = get_compiled()
    in_maps = _pack(inputs)
    res = run_bass_kernel_spmd(nc, in_maps, core_ids=list(range(NCORES)))
    outs = []
    for c in range(NCORES):
        ob = res.results[c]["out"].reshape(128, NT, 4, 64)
        outs.append(ob.transpose(1, 2, 0, 3).reshape(BC, 64)[:, :NL])
    return np.ascontiguousarray(np.concatenate(outs, axis=0))


if __name__ == "__main__":
    rng = np.random.default_rng(0)
    fake = {"x": rng.standard_normal((B, G), dtype=np.float32)}
    print("built", get_compiled())
